# revision 13
# baseline (speedup 1.0000x reference)
"""Trainium2 Bass kernel for DeepSet MLP (embedding-lookup-sum + 3-layer MLP).

Math: u[b] = sum_j W_phi[x[b,j]] + N*b_phi
      y[b] = relu(relu(u@W1+b1)@W2+b2)@W3 + b3

Instead of gathering B*N embedding rows (1 GiB of traffic), each core
computes per-row class histograms and contracts them with the table:
    u = counts @ W_phi,  counts[b,c] = #{j : x[b,j]=c}
The histogram is built on the PE via a class split c = 32*hi + lo:
one-hot H (32 lo-classes) and G (16 hi-classes) per token, then per row
cnt2[b] = H_b^T @ G_b (one matmul per row, j contracted on partitions).
The projection u = cnt2 @ W_phi and the MLP run on the PE as well.

Data-parallel: batch 4096 sharded 512 rows per core across 8 cores.
"""

import os
import numpy as np
from contextlib import ExitStack

STAGE = int(os.environ.get("K_STAGE", "99"))  # debug: stop after stage N

import concourse.bass as bass
import concourse.bacc as bacc
import concourse.tile as tile
import concourse.mybir as mybir
from concourse import masks
from concourse.bass_utils import run_bass_kernel_spmd

B, N, C, PHI = 4096, 512, 512, 128
H1, H2 = 512, 256
NCORES = 8
BS = B // NCORES          # 512 batch rows per core
NB = BS // 128            # 4 batch blocks of 128 rows
NJ = N // 128             # 4 j-chunks
LO, HI = 32, 16           # class split: c = 32*hi + lo

F32 = mybir.dt.float32
BF16 = mybir.dt.bfloat16
I16 = mybir.dt.int16
I32 = mybir.dt.int32
AF = mybir.ActivationFunctionType
ALU = mybir.AluOpType

# number of hi-classes whose one-hot is computed on the ACT engine
# (2 ACT passes each) instead of the DVE (1 pass each) — load balancing.
ACT_EQ_HI = 6


def build_program():
    nc = bacc.Bacc("TRN2", target_bir_lowering=False, debug=False,
                   num_devices=NCORES)

    x32 = nc.dram_tensor("x", [BS, N], I32, kind="ExternalInput")
    wphi = nc.dram_tensor("wphi", [C, PHI], F32, kind="ExternalInput")
    bphi = nc.dram_tensor("bphi", [PHI, 1], F32, kind="ExternalInput")
    w1 = nc.dram_tensor("w1", [PHI, H1], F32, kind="ExternalInput")
    b1 = nc.dram_tensor("b1", [PHI, H1 // PHI], F32, kind="ExternalInput")
    w2 = nc.dram_tensor("w2", [H1, H2], F32, kind="ExternalInput")
    b2 = nc.dram_tensor("b2", [PHI, H2 // PHI], F32, kind="ExternalInput")
    w3 = nc.dram_tensor("w3", [PHI, H2 // PHI], F32, kind="ExternalInput")
    b3 = nc.dram_tensor("b3", [1, 1], F32, kind="ExternalInput")
    out = nc.dram_tensor("out", [1, BS], F32, kind="ExternalOutput")

    with tile.TileContext(nc) as tc:
        with ExitStack() as ctx:
            _emit(ctx, tc, nc, x32, wphi, bphi, w1, b1, w2, b2, w3, b3, out)
    nc.compile()
    return nc


def _emit(ctx, tc, nc, x32, wphi, bphi, w1, b1, w2, b2, w3, b3, out):
    consts = ctx.enter_context(tc.tile_pool(name="consts", bufs=1))
    xin = ctx.enter_context(tc.tile_pool(name="xin", bufs=2))
    xtp = ctx.enter_context(tc.tile_pool(name="xtp", bufs=1))
    eqp = ctx.enter_context(tc.tile_pool(name="eqp", bufs=2))
    fp = ctx.enter_context(tc.tile_pool(name="fp", bufs=1))
    mlp = ctx.enter_context(tc.tile_pool(name="mlp", bufs=1))
    ps_t = ctx.enter_context(tc.tile_pool(name="ps_t", bufs=2, space="PSUM"))
    ps_cnt = ctx.enter_context(tc.tile_pool(name="ps_cnt", bufs=2, space="PSUM"))
    ps_u = ctx.enter_context(tc.tile_pool(name="ps_u", bufs=1, space="PSUM"))
    ps_mlp = ctx.enter_context(tc.tile_pool(name="ps_mlp", bufs=2, space="PSUM"))
    ps_y = ctx.enter_context(tc.tile_pool(name="ps_y", bufs=1, space="PSUM"))

    ident = consts.tile([128, 128], F32)
    masks.make_identity(nc, ident[:])

    # ---- weights / biases to SBUF ----
    # wphiP: W_phi replicated 4x along partitions: partition (i*32+r) holds
    # row W_phi[h*32+r] at free slot h (h=hi class). Each PE matmul for
    # (h, i) uses lhsT = wphiP[32i:32i+32, 128h:128h+128].
    wphiP = consts.tile([128, HI * PHI], F32)
    wsrc = wphi.ap().rearrange("(h r) d -> r h d", r=32)
    for i in range(4):
        nc.sync.dma_start(wphiP[32 * i:32 * (i + 1), :], wsrc)
    # bf16x2 decomposition of the table for exact-ish bf16 matmuls
    wphiH = consts.tile([128, HI * PHI], BF16)
    wphiL = consts.tile([128, HI * PHI], BF16)
    wres = consts.tile([128, HI * PHI], F32)
    nc.vector.tensor_copy(wphiH[:], wphiP[:])
    nc.vector.tensor_tensor(out=wres[:], in0=wphiP[:], in1=wphiH[:],
                            op=ALU.subtract)
    nc.vector.tensor_copy(wphiL[:], wres[:])

    bphi_sb = consts.tile([128, 1], F32)
    nc.sync.dma_start(bphi_sb[:], bphi.ap())
    bphiN = consts.tile([128, 1], F32)
    nc.vector.tensor_scalar(out=bphiN[:], in0=bphi_sb[:], scalar1=float(N),
                            scalar2=None, op0=ALU.mult)

    w1sb = consts.tile([128, H1], F32)
    nc.sync.dma_start(w1sb[:], w1.ap())
    b1sb = consts.tile([128, 4], F32)
    nc.sync.dma_start(b1sb[:], b1.ap())
    w2sb = consts.tile([128, 4 * H2], F32)
    nc.sync.dma_start(w2sb[:], w2.ap().rearrange("(c p) h -> p c h", p=128))
    b2sb = consts.tile([128, 2], F32)
    nc.sync.dma_start(b2sb[:], b2.ap())
    w3sb = consts.tile([128, 2], F32)
    nc.sync.dma_start(w3sb[:], w3.ap())
    b3sb = consts.tile([1, 1], F32)
    nc.sync.dma_start(b3sb[:], b3.ap())

    # per-partition constant biases for the ACT-engine one-hot passes
    cneg = consts.tile([128, ACT_EQ_HI], F32)
    for k in range(ACT_EQ_HI):
        nc.gpsimd.memset(cneg[:, k:k + 1], float(-(HI - ACT_EQ_HI + k)))
    cone = consts.tile([128, 1], F32)
    nc.gpsimd.memset(cone[:], 1.0)

    # ---- index staging: transpose to [j, b] and split classes ----
    xiT = xtp.tile([128, NJ * BS], I16)   # [j, (jc, b)]
    xhiT = xtp.tile([128, NJ * BS], I16)
    xloT = xtp.tile([128, NJ * BS], I16)
    # F: per-row joint counts, partition (i*32+lo), free (hi, b)
    fcnt = fp.tile([128, HI * BS], BF16)

    usb = mlp.tile([128, BS], F32)
    h1sb = [mlp.tile([128, BS], F32, tag=f"h1_{k}", name=f"h1sb{k}")
            for k in range(4)]
    h2sb = [mlp.tile([128, BS], F32, tag=f"h2_{k}", name=f"h2sb{k}")
            for k in range(2)]
    ysb = mlp.tile([1, BS], F32)

    def dbg_out(src_f32_row):
        # debug escape hatch: ship one row to `out` and stop emitting
        nc.vector.tensor_copy(ysb[:], src_f32_row)
        nc.sync.dma_start(out.ap(), ysb[:])

    if STAGE == 0:
        t0 = mlp.tile([1, BS], F32, name="dbg0")
        nc.vector.tensor_copy(t0[:], wphiP[0:1, 0:BS])
        dbg_out(t0[:])
        return

    for bb in range(NB):
        # --- stage A: load 128 rows, cast, transpose, split hi/lo ---
        xrows = xin.tile([128, N], I32, tag="xrows")
        nc.sync.dma_start(xrows[:], x32.ap()[bb * 128:(bb + 1) * 128, :])
        xf = xin.tile([128, N], F32, tag="xf")
        nc.vector.tensor_copy(xf[:], xrows[:])
        for jc in range(NJ):
            pst = ps_t.tile([128, 128], F32)
            nc.tensor.transpose(pst[:], xf[:, jc * 128:(jc + 1) * 128],
                                ident[:])
            col = jc * BS + bb * 128
            nc.vector.tensor_copy(xiT[:, col:col + 128], pst[:])
        xiv = xiT[:].rearrange("p (jc b) -> p jc b", jc=NJ)[
            :, :, bb * 128:(bb + 1) * 128]
        xhv = xhiT[:].rearrange("p (jc b) -> p jc b", jc=NJ)[
            :, :, bb * 128:(bb + 1) * 128]
        xlv = xloT[:].rearrange("p (jc b) -> p jc b", jc=NJ)[
            :, :, bb * 128:(bb + 1) * 128]
        nc.vector.tensor_scalar(out=xhv, in0=xiv, scalar1=5, scalar2=None,
                                op0=ALU.logical_shift_right)
        nc.vector.tensor_scalar(out=xlv, in0=xiv, scalar1=31, scalar2=None,
                                op0=ALU.bitwise_and)
        if STAGE == 1:
            t1 = mlp.tile([1, BS], F32, name="dbg1")
            nc.vector.tensor_copy(t1[:], xloT[0:1, :BS])
            dbg_out(t1[:])
            return

        # --- stage B: one-hots via is_equal ---
        # H2 [j, (b, jc, lo)]  G2 [j, (b, jc, hi)]
        h2t = eqp.tile([128, 128 * NJ * LO], BF16, tag="h2t")
        g2t = eqp.tile([128, 128 * NJ * HI], BF16, tag="g2t")
        h2v = h2t[:].rearrange("p (b jc l) -> p b jc l", b=128, jc=NJ)
        g2v = g2t[:].rearrange("p (b jc h) -> p b jc h", b=128, jc=NJ)
        # eq input views ordered (b, jc) to match output iteration
        xh_b = xhv.transpose([0, 2, 1])
        xl_b = xlv.transpose([0, 2, 1])
        for lo in range(LO):
            nc.vector.tensor_scalar(out=h2v[:, :, :, lo:lo + 1], in0=xl_b,
                                    scalar1=lo, scalar2=None, op0=ALU.is_equal)
        scr = eqp.tile([128, NJ * 128], BF16, tag="scr")
        for hi in range(HI):
            ov = g2v[:, :, :, hi:hi + 1]
            if hi < HI - ACT_EQ_HI:
                nc.vector.tensor_scalar(out=ov, in0=xh_b, scalar1=hi,
                                        scalar2=None, op0=ALU.is_equal)
            else:
                # ACT: t=(x-hi)^2 ; onehot = relu(1-t)
                nc.scalar.activation(scr[:], xh_b, AF.Square,
                                     bias=cneg[:, hi - (HI - ACT_EQ_HI):
                                               hi - (HI - ACT_EQ_HI) + 1],
                                     scale=1.0)
                nc.scalar.activation(
                    ov, scr[:].rearrange("p (b jc) -> p b jc", b=128),
                    AF.Relu, bias=cone[:, 0:1], scale=-1.0)

        if STAGE == 2:
            t2 = mlp.tile([1, BS], F32, name="dbg2")
            nc.vector.tensor_copy(t2[:], h2t[0:1, :BS])
            dbg_out(t2[:])
            return

        # --- stage C: per-row count matmuls ---
        # one matmul per row: lhsT=[j,(jc,lo)] (128 wide), rhs=[j,(jc,hi)]
        # out[(jc,lo),(jc',hi)]; diagonal jc==jc' blocks hold counts.
        for k8 in range(16):            # 16 banks of 8 rows each
            pc = ps_cnt.tile([128, 512], F32)
            for s in range(8):
                b_l = k8 * 8 + s
                nc.tensor.matmul(
                    pc[:, s * 64:(s + 1) * 64],
                    h2v[:, b_l:b_l + 1, :, :],
                    g2v[:, b_l:b_l + 1, :, :],
                    start=True, stop=True)
            # evacuate diagonal blocks to F (bf16, counts are small ints)
            pcv = pc[:].rearrange("p (s i h) -> p s i h", s=8, i=NJ)
            fv = fcnt[:].rearrange("p (h b) -> p h b", h=HI)
            for i in range(NJ):
                src = pcv[32 * i:32 * (i + 1), :, i:i + 1, :]
                dst = fv[32 * i:32 * (i + 1), :,
                         bb * 128 + k8 * 8:bb * 128 + (k8 + 1) * 8]
                nc.scalar.copy(dst.transpose([0, 2, 1]), src)

    if STAGE == 3:
        t3 = mlp.tile([1, BS], F32, name="dbg3")
        nc.vector.tensor_copy(t3[:], fcnt[0:1, :BS])
        dbg_out(t3[:])
        return

    # ---- projection u_T[d, b] = sum_c counts_T[c, b] * W_phi[c, d] ----
    # Weights are replicated across the 4 partition blocks of wphiH/L, so a
    # single k=128 matmul per hi-class sums over both lo-classes and the 4
    # j-chunk partials in F.
    pu = ps_u.tile([128, BS], F32)
    fv = fcnt[:].rearrange("p (h b) -> p h b", h=HI)
    k = 0
    for h in range(HI):
        for w in (wphiH, wphiL):
            nc.tensor.matmul(
                pu[:], w[:, PHI * h:PHI * (h + 1)], fv[:, h, :],
                start=(k == 0), stop=(k == 2 * HI - 1))
            k += 1
    nc.vector.tensor_scalar(out=usb[:], in0=pu[:], scalar1=bphiN[:, 0:1],
                            scalar2=None, op0=ALU.add)
    if STAGE == 4:
        dbg_out(usb[0:1, :])
        return

    # ---- MLP ----
    for hc in range(4):
        ph = ps_mlp.tile([128, BS], F32, tag="ph", name="ph_a")
        nc.tensor.matmul(ph[:], w1sb[:, hc * 128:(hc + 1) * 128], usb[:],
                         start=True, stop=True)
        nc.scalar.activation(h1sb[hc][:], ph[:], AF.Relu,
                             bias=b1sb[:, hc:hc + 1], scale=1.0)
    w2v = w2sb[:].rearrange("p (c h) -> p c h", c=4)
    for mc in range(2):
        ph = ps_mlp.tile([128, BS], F32, tag="ph", name="ph_b")
        for kc in range(4):
            nc.tensor.matmul(ph[:], w2v[:, kc, mc * 128:(mc + 1) * 128],
                             h1sb[kc][:], start=(kc == 0), stop=(kc == 3))
        nc.scalar.activation(h2sb[mc][:], ph[:], AF.Relu,
                             bias=b2sb[:, mc:mc + 1], scale=1.0)
    py = ps_y.tile([1, BS], F32)
    for kc in range(2):
        nc.tensor.matmul(py[:], w3sb[:, kc:kc + 1], h2sb[kc][:],
                         start=(kc == 0), stop=(kc == 1))
    nc.vector.tensor_scalar(out=ysb[:], in0=py[:], scalar1=b3sb[0:1, 0:1],
                            scalar2=None, op0=ALU.add)
    nc.sync.dma_start(out.ap(), ysb[:])


_CACHED_NC = None


def _get_nc():
    global _CACHED_NC
    if _CACHED_NC is None:
        _CACHED_NC = build_program()
    return _CACHED_NC


def _prep_in_maps(x, W_phi, b_phi, W1, b1, W2, b2, W3, b3):
    x = np.ascontiguousarray(np.asarray(x, dtype=np.int64).astype(np.int32))
    W_phi = np.asarray(W_phi, dtype=np.float32)
    W1 = np.asarray(W1, dtype=np.float32)
    W2 = np.asarray(W2, dtype=np.float32)
    shared = {
        "wphi": W_phi,
        "bphi": np.asarray(b_phi, dtype=np.float32).reshape(PHI, 1),
        "w1": W1,
        "b1": np.ascontiguousarray(
            np.asarray(b1, np.float32).reshape(4, 128).T),
        "w2": W2,
        "b2": np.ascontiguousarray(
            np.asarray(b2, np.float32).reshape(2, 128).T),
        "w3": np.ascontiguousarray(
            np.asarray(W3, np.float32).reshape(2, 128).T),
        "b3": np.asarray(b3, np.float32).reshape(1, 1),
    }
    return [dict(shared, x=np.ascontiguousarray(x[c * BS:(c + 1) * BS]))
            for c in range(NCORES)]


def run(trace=False, **inputs):
    nc = _get_nc()
    in_maps = _prep_in_maps(**inputs)
    res = run_bass_kernel_spmd(nc, in_maps, core_ids=list(range(NCORES)),
                               trace=trace)
    y = np.concatenate([np.asarray(res.results[c]["out"]).reshape(BS)
                        for c in range(NCORES)])
    return y.reshape(B, 1).astype(np.float32), res


def kernel(**inputs):
    y, _ = run(trace=False, **inputs)
    return y
